# revision 34
# baseline (speedup 1.0000x reference)
"""Decode-attention kernel for Trainium2 (8 NeuronCores, tensor-parallel over heads).

Computes, for B=16 single-token queries over an L=4096 KV cache with 16 heads
of dim 128:
    q,k,v = x @ W{q,k,v}.T + b;  cache[current_pos] = k,v (new token)
    out   = softmax(q K^T / sqrt(d)) V @ W_o.T + b_o

Sharding: 2 heads per core. Each core computes its heads' QKV projection,
attention over its slice of the KV cache, and a partial output projection
(w_o column slice); the host sums the 8 partials. All weight/cache tensors are
pre-arranged on the host so every device DMA is a natural-layout (contiguous
per partition) load.

The batch dimension is processed in two interleaved groups so the K loads of
group B stream while group A runs softmax/attn@V — the DMA ring (the
bottleneck; ~137 MB/core) stays busy end to end.
"""

import numpy as np

P = 128  # partitions == head dim

_CACHE: dict = {}


def build_nc(B=16, H=2048, HC=256, L=4096, NHL=2, n_devices=8, mm_dtype="f32r",
             repeat=1):
    import concourse.mybir as mybir
    import concourse.tile as tile
    from concourse import bacc
    from concourse.masks import make_identity

    f32 = mybir.dt.float32
    # float32r streams 1 col/cycle on the PE for N>=256 (vs 4 for fp32),
    # with relaxed (TF32-like) product precision; same 4-byte layout.
    mmdt = mybir.dt.float32r if mm_dtype == "f32r" else f32
    Act = mybir.ActivationFunctionType
    PAIRS = B * NHL           # (b, h) pairs, p = 2*b + h
    KCH = H // P              # contraction chunks for projections
    SUP = min(512, L)         # scores superchunk (one PSUM bank)
    NSUP = L // SUP
    NCH = L // P              # l-chunks for attn@V
    NCHH = NCH // 2           # per half-batch V tile
    NO = H // 512             # output projection N-tiles
    G = 2                     # interleaved batch groups
    BG = B // G
    PG = BG * NHL             # pairs per group
    JH = 2 if NSUP >= 2 else 1  # K l-halves per (b, h)
    NSUPH = NSUP // JH
    LH = L // JH
    assert HC == NHL * P and PG <= 128

    nc = bacc.Bacc(
        "TRN2",
        target_bir_lowering=False,
        debug=False,
        enable_asserts=False,
        num_devices=n_devices,
    )
    xT = nc.dram_tensor("xT", [H, B], f32, kind="ExternalInput").ap()
    wqT = nc.dram_tensor("wqT", [P, KCH, HC], f32, kind="ExternalInput").ap()
    wkT = nc.dram_tensor("wkT", [P, KCH, HC], f32, kind="ExternalInput").ap()
    wvT = nc.dram_tensor("wvT", [P, KCH, HC], f32, kind="ExternalInput").ap()
    woT = nc.dram_tensor("woT", [HC, H], mmdt, kind="ExternalInput").ap()
    bq = nc.dram_tensor("bq", [P, NHL], f32, kind="ExternalInput").ap()
    bk = nc.dram_tensor("bk", [P, NHL], f32, kind="ExternalInput").ap()
    bv = nc.dram_tensor("bv", [P, NHL], f32, kind="ExternalInput").ap()
    bo = nc.dram_tensor("bo", [1, H], mmdt, kind="ExternalInput").ap()
    kT = nc.dram_tensor("kT", [B, HC, L], mmdt, kind="ExternalInput").ap()
    v = nc.dram_tensor("v", [B, 2, P, NCHH, HC], mmdt, kind="ExternalInput").ap()
    mask = nc.dram_tensor("mask", [1, L], mmdt, kind="ExternalInput").ap()
    out = nc.dram_tensor("out", [B, H], f32, kind="ExternalOutput").ap()

    inv = float(1.0 / np.sqrt(P))

    with tile.TileContext(nc) as tc:
        def emit_body():
            with (
                tc.tile_pool(name="pers", bufs=1) as pers,
                tc.tile_pool(name="work", bufs=2) as work,
                tc.tile_pool(name="kpool", bufs=3) as kpool,
                tc.tile_pool(name="vpool", bufs=4) as vpool,
            ):
                ident = pers.tile([P, P], f32)
                make_identity(nc, ident)
                ones_col = pers.tile([P, 1], f32)
                nc.vector.memset(ones_col, 1.0)
                ones_1p = pers.tile([1, P], f32)
                nc.vector.memset(ones_1p, 1.0)
                ones_r = pers.tile([1, P], mmdt)
                nc.vector.tensor_copy(ones_r, ones_1p)
                xT_sb = pers.tile([P, KCH, B], f32)
                nc.sync.dma_start(xT_sb, xT.rearrange("(n p) b -> p n b", p=P))
                bq_sb = pers.tile([P, NHL], f32)
                nc.sync.dma_start(bq_sb, bq)
                bk_sb = pers.tile([P, NHL], f32)
                nc.sync.dma_start(bk_sb, bk)
                bv_sb = pers.tile([P, NHL], f32)
                nc.sync.dma_start(bv_sb, bv)
                bo_sb = pers.tile([1, H], mmdt)
                nc.sync.dma_start(bo_sb, bo)
                mask_sb = pers.tile([1, L], mmdt)
                nc.sync.dma_start(mask_sb, mask)
                wo_sb = pers.tile([P, NHL, H], mmdt)
                nc.sync.dma_start(wo_sb, woT.rearrange("(h p) m -> p h m", p=P))

                qT_pairs = pers.tile([P, B, NHL], f32)
                kT_pairs = pers.tile([P, B, NHL], f32)
                vT_pairs = pers.tile([P, B, NHL], f32)

                # ---- phase 1: QKV projections (per local head) ----
                snew = []
                with (
                    tc.tile_pool(name="wpool", bufs=1) as wpool,
                    tc.tile_pool(name="pp1", bufs=2, space="PSUM") as pp1,
                ):
                    for wdram, bias_sb, dest, scale in (
                        (wqT, bq_sb, qT_pairs, inv),
                        (wkT, bk_sb, kT_pairs, 1.0),
                        (wvT, bv_sb, vT_pairs, 1.0),
                    ):
                        w_sb = wpool.tile([P, KCH, HC], f32, tag="w", name="w_sb")
                        nc.sync.dma_start(w_sb, wdram)
                        for h in range(NHL):
                            ps = pp1.tile([P, B], f32, tag="psproj", name="ps_proj")
                            for n in range(KCH):
                                nc.tensor.matmul(
                                    ps,
                                    w_sb[:, n, h * P : (h + 1) * P],
                                    xT_sb[:, n],
                                    start=(n == 0),
                                    stop=(n == KCH - 1),
                                )
                            nc.scalar.activation(
                                dest[:, :, h], ps, Act.Identity,
                                bias=bias_sb[:, h : h + 1], scale=scale,
                            )

                    # s_new[p] = q_scaled . k_new per pair (PE dot via ones)
                    prod = work.tile([P, B, NHL], f32)
                    nc.vector.tensor_mul(prod, qT_pairs, kT_pairs)
                    prod2 = prod.rearrange("p b h -> p (b h)")
                    for g in range(G):
                        sn_ps = pp1.tile([PG, 1], f32, tag="psnew", name="sn_ps")
                        nc.tensor.matmul(
                            sn_ps, prod2[:, g * PG : (g + 1) * PG], ones_col,
                            start=True, stop=True,
                        )
                        sn = pers.tile([PG, 1], f32, name=f"snew{g}")
                        nc.vector.tensor_copy(sn, sn_ps)
                        snew.append(sn)

                # qdiag per group: [P, PG] with only column p_local nonzero
                qp2 = qT_pairs.rearrange("p b h -> p (b h)")
                qdiag = []
                for g in range(G):
                    qd = pers.tile([P, PG, PG], mmdt, name=f"qdiag{g}")
                    qz = work.tile([P, PG, PG], f32, tag="qdz", name="qz")
                    nc.vector.memset(qz, 0.0)
                    nc.vector.tensor_copy(qd, qz)
                    for pl in range(PG):
                        nc.vector.tensor_copy(
                            qd[:, pl, pl : pl + 1], qp2[:, g * PG + pl : g * PG + pl + 1]
                        )
                    qdiag.append(qd)

                aoT = pers.tile([P, B, NHL], mmdt)

                with tc.tile_pool(name="pp", bufs=1, space="PSUM") as pp:
                    for g in range(G):
                        # ---- scores for this group's pairs ----
                        scores_g = pers.tile([PG, L], f32, name=f"scores{g}")
                        for jh in range(JH):
                            sc_ps = [
                                pp.tile([PG, SUP], f32, tag="psc", bufs=NSUPH,
                                        name=f"sc_ps{g}_{jh}_{jj}")
                                for jj in range(NSUPH)
                            ]
                            for jj in range(NSUPH):
                                j = jh * NSUPH + jj
                                nc.tensor.matmul(
                                    sc_ps[jj], ones_r[:, :PG],
                                    mask_sb[:, j * SUP : (j + 1) * SUP],
                                    start=True, stop=False,
                                )
                            for bl in range(BG):
                                b = g * BG + bl
                                for h in range(NHL):
                                    pl = 2 * bl + h
                                    kt = kpool.tile([P, LH], mmdt, tag="kt", name="kt")
                                    nc.sync.dma_start(
                                        kt,
                                        kT[b, h * P : (h + 1) * P,
                                           jh * LH : (jh + 1) * LH],
                                    )
                                    for jj in range(NSUPH):
                                        nc.tensor.matmul(
                                            sc_ps[jj],
                                            qdiag[g][:, pl],
                                            kt[:, jj * SUP : (jj + 1) * SUP],
                                            start=False,
                                            stop=(pl == PG - 1),
                                        )
                            for jj in range(NSUPH):
                                j = jh * NSUPH + jj
                                nc.vector.tensor_copy(
                                    scores_g[:, j * SUP : (j + 1) * SUP], sc_ps[jj]
                                )

                        # ---- V loads for this group (program-order here so the
                        # SP ring streams K_g, V_g, K_g+1, V_g+1 back to back) ----
                        vts = []
                        for bl in range(BG):
                            b = g * BG + bl
                            for u in range(2):
                                vt = vpool.tile([P, NCHH, HC], mmdt, tag="vt", name="vt")
                                nc.sync.dma_start(vt, v[b, u])
                                vts.append(vt)

                        # ---- softmax over l (rows = group pairs) ----
                        m0n = work.tile([PG, 1], f32, tag="m0n", name="m0n")
                        nc.vector.tensor_reduce(
                            m0n, scores_g, axis=mybir.AxisListType.X,
                            op=mybir.AluOpType.max, negate=True,
                        )
                        nsnew = work.tile([PG, 1], f32, tag="nsnew", name="nsnew")
                        nc.vector.tensor_scalar_mul(nsnew, snew[g], -1.0)
                        bias_t = work.tile([PG, 1], f32, tag="bias_t", name="bias_t")
                        nc.vector.tensor_tensor(
                            bias_t, m0n, nsnew, op=mybir.AluOpType.min
                        )
                        sum1 = work.tile([PG, 1], f32, tag="sum1", name="sum1")
                        # in-place exp; masked cols (-1e30) become 0 and the
                        # fused accum gives the softmax denominator
                        nc.scalar.activation(
                            scores_g, scores_g, Act.Exp, bias=bias_t, accum_out=sum1
                        )
                        anew = work.tile([PG, 1], f32, tag="anew", name="anew")
                        nc.scalar.activation(anew, snew[g], Act.Exp, bias=bias_t)
                        total = work.tile([PG, 1], f32, tag="total", name="total")
                        nc.vector.tensor_add(total, sum1, anew)
                        recip = work.tile([PG, 1], f32, tag="recip", name="recip")
                        nc.vector.reciprocal(recip, total)

                        # broadcast anew / recip across all 128 partitions
                        bcA = pers.tile([P, BG, NHL], f32, name=f"bcA{g}")
                        bcR = pers.tile([P, BG, NHL], f32, name=f"bcR{g}")
                        for src, dst in ((anew, bcA), (recip, bcR)):
                            rps = pp.tile([P, PG], f32, tag="trp", bufs=2, name="rps")
                            nc.tensor.transpose(
                                rps[0:1, :], src, ident[:PG, :PG]
                            )
                            row_sb = work.tile([1, PG], f32, tag="rowsb", name="row_sb")
                            nc.vector.tensor_copy(row_sb, rps[0:1, :])
                            bc_ps = pp.tile([P, PG], f32, tag="trp", bufs=2, name="bc_ps")
                            nc.tensor.matmul(bc_ps, ones_1p, row_sb, start=True, stop=True)
                            nc.vector.tensor_copy(
                                dst.rearrange("p b h -> p (b h)"), bc_ps
                            )

                        # ---- attn^T ----
                        attnT = pers.tile([P, NCH, PG], mmdt, name=f"attnT{g}")
                        for n in range(NCH):
                            tps = pp.tile([P, PG], f32, tag="trp", bufs=2, name="tps")
                            nc.tensor.transpose(
                                tps, scores_g[:, n * P : (n + 1) * P], ident[:PG, :PG]
                            )
                            nc.vector.tensor_copy(attnT[:, n], tps)

                        # ---- attn @ V per batch ----
                        for bl in range(BG):
                            b = g * BG + bl
                            ps = pp.tile([NHL, HC], f32, tag="av", bufs=2, name="ps_av")
                            for n in range(NCH):
                                nc.tensor.matmul(
                                    ps,
                                    attnT[:, n, 2 * bl : 2 * bl + 2],
                                    vts[2 * bl + n // NCHH][:, n % NCHH],
                                    start=(n == 0),
                                    stop=(n == NCH - 1),
                                )
                            sbb = work.tile([NHL, HC], f32, tag="sbb", name="sbb")
                            nc.vector.tensor_copy(sbb, ps)
                            for h in range(NHL):
                                tp = pp.tile([P, PG], f32, tag="trp", bufs=2, name="tp")
                                nc.tensor.transpose(
                                    tp[:, :NHL], sbb[:, h * P : (h + 1) * P],
                                    ident[:NHL, :NHL],
                                )
                                nc.vector.tensor_copy(
                                    aoT[:, b, h : h + 1], tp[:, h : h + 1]
                                )

                        # new-token V correction + 1/sum
                        bsl = slice(g * BG, (g + 1) * BG)
                        for h in range(NHL):
                            tmp = work.tile([P, BG], f32, tag="corr", name="tmp")
                            nc.vector.tensor_mul(tmp, vT_pairs[:, bsl, h], bcA[:, :, h])
                            nc.vector.tensor_add(
                                aoT[:, bsl, h], aoT[:, bsl, h], tmp
                            )
                            nc.vector.tensor_mul(
                                aoT[:, bsl, h], aoT[:, bsl, h], bcR[:, :, h]
                            )

                        # ---- per-group output projection + bias (each output row
                        # belongs to exactly one group, so + b_o lands once) ----
                        out_g = pers.tile([BG, H], f32, name=f"out_sb{g}")
                        for no in range(NO):
                            pso = pp.tile([BG, 512], f32, tag="av", bufs=2, name="pso")
                            for h in range(NHL):
                                nc.tensor.matmul(
                                    pso,
                                    aoT[:, bsl, h],
                                    wo_sb[:, h, no * 512 : (no + 1) * 512],
                                    start=(h == 0),
                                    stop=False,
                                )
                            nc.tensor.matmul(
                                pso, ones_r[:, :BG],
                                bo_sb[:, no * 512 : (no + 1) * 512],
                                start=False, stop=True,
                            )
                            nc.vector.tensor_copy(
                                out_g[:, no * 512 : (no + 1) * 512], pso
                            )
                        nc.sync.dma_start(out[g * BG : (g + 1) * BG, :], out_g)


        for _ in range(repeat):
            emit_body()

    nc.compile()
    return nc


def make_core_inputs(x, k_cache, v_cache, w_q, w_k, w_v, w_o, b_q, b_k, b_v, b_o,
                     current_pos, n_cores=8):
    """Host-side shard + layout prep. Returns list of per-core input dicts."""
    B, S, H = x.shape
    L = k_cache.shape[1]
    cp = int(current_pos)
    HC = H // n_cores
    NHL = HC // P
    inv = 1.0 / np.sqrt(P)

    x2 = np.ascontiguousarray(x.reshape(B, H).T, dtype=np.float32)  # [H, B]
    KCH = H // P
    NCHH = (L // P) // 2

    def wshuf(w):
        # [HC, H] weight slice -> [P, KCH, HC]: one contiguous 16 KB run per
        # partition for the projection-weight DMA
        return np.ascontiguousarray(
            np.asarray(w).T.reshape(KCH, P, HC).transpose(1, 0, 2)
        )

    def vshuf(vc):
        # [B, L, HC] -> [B, 2, P, NCHH, HC] with element (b,u,p,n,c) =
        # vc[b, (u*NCHH + n)*P + p, c]: per-partition contiguous V DMA runs
        return np.ascontiguousarray(
            np.asarray(vc).reshape(B, 2, NCHH, P, HC).transpose(0, 1, 3, 2, 4)
        )
    kT_full = np.ascontiguousarray(k_cache.transpose(0, 2, 1))      # [B, H, L]
    maskvec = np.zeros((1, L), dtype=np.float32)
    maskvec[0, cp:] = -1e30

    maps = []
    for c in range(n_cores):
        r = slice(c * HC, (c + 1) * HC)
        m = {
            "xT": x2,
            "wqT": wshuf(w_q[r, :]),
            "wkT": wshuf(w_k[r, :]),
            "wvT": wshuf(w_v[r, :]),
            "woT": np.ascontiguousarray(w_o[:, r].T),
            "bq": np.ascontiguousarray((b_q[r] * inv).reshape(NHL, P).T),
            "bk": np.ascontiguousarray(b_k[r].reshape(NHL, P).T),
            "bv": np.ascontiguousarray(b_v[r].reshape(NHL, P).T),
            "bo": (b_o if c == 0 else np.zeros_like(b_o)).reshape(1, H),
            "kT": np.ascontiguousarray(kT_full[:, r, :]),
            "v": vshuf(v_cache[:, :, r]),
            "mask": maskvec,
        }
        maps.append({k: np.asarray(a, dtype=np.float32) for k, a in m.items()})
    return maps


def kernel(x, k_cache, v_cache, w_q, w_k, w_v, w_o, b_q, b_k, b_v, b_o, current_pos):
    from concourse import bass_utils

    x, k_cache, v_cache = np.asarray(x), np.asarray(k_cache), np.asarray(v_cache)
    w_q, w_k, w_v, w_o = (np.asarray(a) for a in (w_q, w_k, w_v, w_o))
    b_q, b_k, b_v, b_o = (np.asarray(a) for a in (b_q, b_k, b_v, b_o))
    B, S, H = x.shape
    n_cores = 8
    key = (B, H, k_cache.shape[1])
    if key not in _CACHE:
        _CACHE[key] = build_nc(
            B=B, H=H, HC=H // n_cores, L=k_cache.shape[1],
            NHL=(H // n_cores) // P, n_devices=n_cores,
        )
    nc = _CACHE[key]
    in_maps = make_core_inputs(
        x, k_cache, v_cache, w_q, w_k, w_v, w_o, b_q, b_k, b_v, b_o, current_pos,
        n_cores=n_cores,
    )
    res = bass_utils.run_bass_kernel_spmd(nc, in_maps, core_ids=list(range(n_cores)))
    total = np.zeros((B, H), dtype=np.float32)
    for r in res.results:
        total += r["out"]
    return total.reshape(B, S, H).astype(np.float32)



# revision 36
# speedup vs baseline: 1.0102x; 1.0102x over previous
"""Decode-attention kernel for Trainium2 (8 NeuronCores, tensor-parallel over heads).

Computes, for B=16 single-token queries over an L=4096 KV cache with 16 heads
of dim 128:
    q,k,v = x @ W{q,k,v}.T + b;  cache[current_pos] = k,v (new token)
    out   = softmax(q K^T / sqrt(d)) V @ W_o.T + b_o

Sharding: 2 heads per core. Each core computes its heads' QKV projection,
attention over its slice of the KV cache, and a partial output projection
(w_o column slice); the host sums the 8 partials. All weight/cache tensors are
pre-arranged on the host so every device DMA is a natural-layout (contiguous
per partition) load.

The batch dimension is processed in two interleaved groups so the K loads of
group B stream while group A runs softmax/attn@V — the DMA ring (the
bottleneck; ~137 MB/core) stays busy end to end.
"""

import numpy as np

P = 128  # partitions == head dim

_CACHE: dict = {}


def build_nc(B=16, H=2048, HC=256, L=4096, NHL=2, n_devices=8, mm_dtype="f32r",
             repeat=1):
    import concourse.mybir as mybir
    import concourse.tile as tile
    from concourse import bacc
    from concourse.masks import make_identity

    f32 = mybir.dt.float32
    # float32r streams 1 col/cycle on the PE for N>=256 (vs 4 for fp32),
    # with relaxed (TF32-like) product precision; same 4-byte layout.
    mmdt = mybir.dt.float32r if mm_dtype == "f32r" else f32
    Act = mybir.ActivationFunctionType
    PAIRS = B * NHL           # (b, h) pairs, p = 2*b + h
    KCH = H // P              # contraction chunks for projections
    SUP = min(512, L)         # scores superchunk (one PSUM bank)
    NSUP = L // SUP
    NCH = L // P              # l-chunks for attn@V
    NCHH = NCH // 2           # per half-batch V tile
    NO = H // 512             # output projection N-tiles
    G = 2                     # interleaved batch groups
    BG = B // G
    PG = BG * NHL             # pairs per group
    JH = 2 if NSUP >= 2 else 1  # K l-halves per (b, h)
    NSUPH = NSUP // JH
    LH = L // JH
    assert HC == NHL * P and PG <= 128

    nc = bacc.Bacc(
        "TRN2",
        target_bir_lowering=False,
        debug=False,
        enable_asserts=False,
        num_devices=n_devices,
    )
    xT = nc.dram_tensor("xT", [H, B], f32, kind="ExternalInput").ap()
    wqT = nc.dram_tensor("wqT", [P, KCH, HC], f32, kind="ExternalInput").ap()
    wkT = nc.dram_tensor("wkT", [P, KCH, HC], f32, kind="ExternalInput").ap()
    wvT = nc.dram_tensor("wvT", [P, KCH, HC], f32, kind="ExternalInput").ap()
    woT = nc.dram_tensor("woT", [HC, H], mmdt, kind="ExternalInput").ap()
    bq = nc.dram_tensor("bq", [P, NHL], f32, kind="ExternalInput").ap()
    bk = nc.dram_tensor("bk", [P, NHL], f32, kind="ExternalInput").ap()
    bv = nc.dram_tensor("bv", [P, NHL], f32, kind="ExternalInput").ap()
    bo = nc.dram_tensor("bo", [1, H], mmdt, kind="ExternalInput").ap()
    kT = nc.dram_tensor("kT", [B, HC, L], mmdt, kind="ExternalInput").ap()
    v = nc.dram_tensor("v", [B, 2, P, NCHH, HC], mmdt, kind="ExternalInput").ap()
    mask = nc.dram_tensor("mask", [1, L], mmdt, kind="ExternalInput").ap()
    out = nc.dram_tensor("out", [B, H], f32, kind="ExternalOutput").ap()

    inv = float(1.0 / np.sqrt(P))

    with tile.TileContext(nc) as tc:
        def emit_body():
            with (
                tc.tile_pool(name="pers", bufs=1) as pers,
                tc.tile_pool(name="work", bufs=2) as work,
                tc.tile_pool(name="kpool", bufs=3) as kpool,
                tc.tile_pool(name="vpool", bufs=4) as vpool,
            ):
                ident = pers.tile([P, P], f32)
                make_identity(nc, ident)
                ones_col = pers.tile([P, 1], f32)
                nc.vector.memset(ones_col, 1.0)
                ones_1p = pers.tile([1, P], f32)
                nc.vector.memset(ones_1p, 1.0)
                ones_r = pers.tile([1, P], mmdt)
                nc.vector.tensor_copy(ones_r, ones_1p)
                xT_sb = pers.tile([P, KCH, B], f32)
                nc.sync.dma_start(xT_sb, xT.rearrange("(n p) b -> p n b", p=P))
                bq_sb = pers.tile([P, NHL], f32)
                nc.sync.dma_start(bq_sb, bq)
                bk_sb = pers.tile([P, NHL], f32)
                nc.sync.dma_start(bk_sb, bk)
                bv_sb = pers.tile([P, NHL], f32)
                nc.sync.dma_start(bv_sb, bv)
                bo_sb = pers.tile([1, H], mmdt)
                nc.sync.dma_start(bo_sb, bo)
                mask_sb = pers.tile([1, L], mmdt)
                nc.sync.dma_start(mask_sb, mask)
                wo_sb = pers.tile([P, NHL, H], mmdt)
                nc.sync.dma_start(wo_sb, woT.rearrange("(h p) m -> p h m", p=P))

                qT_pairs = pers.tile([P, B, NHL], f32)
                kT_pairs = pers.tile([P, B, NHL], f32)
                vT_pairs = pers.tile([P, B, NHL], f32)

                # ---- phase 1: QKV projections (per local head) ----
                snew = []
                with (
                    tc.tile_pool(name="wpool", bufs=1) as wpool,
                    tc.tile_pool(name="pp1", bufs=2, space="PSUM") as pp1,
                ):
                    for wdram, bias_sb, dest, scale in (
                        (wqT, bq_sb, qT_pairs, inv),
                        (wkT, bk_sb, kT_pairs, 1.0),
                        (wvT, bv_sb, vT_pairs, 1.0),
                    ):
                        w_sb = wpool.tile([P, KCH, HC], f32, tag="w", name="w_sb")
                        nc.sync.dma_start(w_sb, wdram)
                        for h in range(NHL):
                            ps = pp1.tile([P, B], f32, tag="psproj", name="ps_proj")
                            for n in range(KCH):
                                nc.tensor.matmul(
                                    ps,
                                    w_sb[:, n, h * P : (h + 1) * P],
                                    xT_sb[:, n],
                                    start=(n == 0),
                                    stop=(n == KCH - 1),
                                )
                            nc.scalar.activation(
                                dest[:, :, h], ps, Act.Identity,
                                bias=bias_sb[:, h : h + 1], scale=scale,
                            )

                    # s_new[p] = q_scaled . k_new per pair (PE dot via ones)
                    prod = work.tile([P, B, NHL], f32)
                    nc.vector.tensor_mul(prod, qT_pairs, kT_pairs)
                    prod2 = prod.rearrange("p b h -> p (b h)")
                    for g in range(G):
                        sn_ps = pp1.tile([PG, 1], f32, tag="psnew", name="sn_ps")
                        nc.tensor.matmul(
                            sn_ps, prod2[:, g * PG : (g + 1) * PG], ones_col,
                            start=True, stop=True,
                        )
                        sn = pers.tile([PG, 1], f32, name=f"snew{g}")
                        nc.vector.tensor_copy(sn, sn_ps)
                        snew.append(sn)

                # qdiag per group: [P, PG] with only column p_local nonzero
                qp2 = qT_pairs.rearrange("p b h -> p (b h)")
                qdiag = []
                for g in range(G):
                    qd = pers.tile([P, PG, PG], mmdt, name=f"qdiag{g}")
                    qz = work.tile([P, PG, PG], f32, tag="qdz", name="qz")
                    nc.vector.memset(qz, 0.0)
                    nc.vector.tensor_copy(qd, qz)
                    for pl in range(PG):
                        nc.vector.tensor_copy(
                            qd[:, pl, pl : pl + 1], qp2[:, g * PG + pl : g * PG + pl + 1]
                        )
                    qdiag.append(qd)

                aoT = pers.tile([P, B, NHL], mmdt)

                with tc.tile_pool(name="pp", bufs=1, space="PSUM") as pp:
                    for g in range(G):
                        # ---- scores for this group's pairs ----
                        scores_g = pers.tile([PG, L], f32, name=f"scores{g}")
                        for jh in range(JH):
                            sc_ps = [
                                pp.tile([PG, SUP], f32, tag="psc", bufs=NSUPH,
                                        name=f"sc_ps{g}_{jh}_{jj}")
                                for jj in range(NSUPH)
                            ]
                            for jj in range(NSUPH):
                                j = jh * NSUPH + jj
                                nc.tensor.matmul(
                                    sc_ps[jj], ones_r[:, :PG],
                                    mask_sb[:, j * SUP : (j + 1) * SUP],
                                    start=True, stop=False,
                                )
                            for bl in range(BG):
                                b = g * BG + bl
                                for h in range(NHL):
                                    pl = 2 * bl + h
                                    kt = kpool.tile([P, LH], mmdt, tag="kt", name="kt")
                                    nc.sync.dma_start(
                                        kt,
                                        kT[b, h * P : (h + 1) * P,
                                           jh * LH : (jh + 1) * LH],
                                    )
                                    for jj in range(NSUPH):
                                        nc.tensor.matmul(
                                            sc_ps[jj],
                                            qdiag[g][:, pl],
                                            kt[:, jj * SUP : (jj + 1) * SUP],
                                            start=False,
                                            stop=(pl == PG - 1),
                                        )
                            for jj in range(NSUPH):
                                j = jh * NSUPH + jj
                                nc.vector.tensor_copy(
                                    scores_g[:, j * SUP : (j + 1) * SUP], sc_ps[jj]
                                )

                        # ---- V loads for this group (program-order here so the
                        # SP ring streams K_g, V_g, K_g+1, V_g+1 back to back) ----
                        vts = []
                        for bl in range(BG):
                            b = g * BG + bl
                            for u in range(2):
                                vt = vpool.tile([P, NCHH, HC], mmdt, tag="vt", name="vt")
                                nc.sync.dma_start(vt, v[b, u])
                                vts.append(vt)

                        # ---- softmax over l (rows = group pairs) ----
                        m0n = work.tile([PG, 1], f32, tag="m0n", name="m0n")
                        nc.vector.tensor_reduce(
                            m0n, scores_g, axis=mybir.AxisListType.X,
                            op=mybir.AluOpType.max, negate=True,
                        )
                        nsnew = work.tile([PG, 1], f32, tag="nsnew", name="nsnew")
                        nc.vector.tensor_scalar_mul(nsnew, snew[g], -1.0)
                        bias_t = work.tile([PG, 1], f32, tag="bias_t", name="bias_t")
                        nc.vector.tensor_tensor(
                            bias_t, m0n, nsnew, op=mybir.AluOpType.min
                        )
                        sum1 = work.tile([PG, 1], f32, tag="sum1", name="sum1")
                        # in-place exp; masked cols (-1e30) become 0 and the
                        # fused accum gives the softmax denominator
                        nc.scalar.activation(
                            scores_g, scores_g, Act.Exp, bias=bias_t, accum_out=sum1
                        )
                        anew = work.tile([PG, 1], f32, tag="anew", name="anew")
                        nc.scalar.activation(anew, snew[g], Act.Exp, bias=bias_t)
                        total = work.tile([PG, 1], f32, tag="total", name="total")
                        nc.vector.tensor_add(total, sum1, anew)
                        recip = work.tile([PG, 1], f32, tag="recip", name="recip")
                        nc.vector.reciprocal(recip, total)

                        # broadcast anew / recip across all 128 partitions
                        bcA = pers.tile([P, BG, NHL], f32, name=f"bcA{g}")
                        bcR = pers.tile([P, BG, NHL], f32, name=f"bcR{g}")
                        for src, dst in ((anew, bcA), (recip, bcR)):
                            rps = pp.tile([P, PG], f32, tag="trp", bufs=2, name="rps")
                            nc.tensor.transpose(
                                rps[0:1, :], src, ident[:PG, :PG]
                            )
                            row_sb = work.tile([1, PG], f32, tag="rowsb", name="row_sb")
                            nc.vector.tensor_copy(row_sb, rps[0:1, :])
                            bc_ps = pp.tile([P, PG], f32, tag="trp", bufs=2, name="bc_ps")
                            nc.tensor.matmul(bc_ps, ones_1p, row_sb, start=True, stop=True)
                            nc.vector.tensor_copy(
                                dst.rearrange("p b h -> p (b h)"), bc_ps
                            )

                        # ---- attn^T ----
                        attnT = pers.tile([P, NCH, PG], mmdt, name=f"attnT{g}")
                        for n in range(NCH):
                            tps = pp.tile([P, PG], f32, tag="trp", bufs=2, name="tps")
                            nc.tensor.transpose(
                                tps, scores_g[:, n * P : (n + 1) * P], ident[:PG, :PG]
                            )
                            nc.vector.tensor_copy(attnT[:, n], tps)

                        # ---- attn @ V per batch ----
                        for bl in range(BG):
                            b = g * BG + bl
                            ps = pp.tile([NHL, HC], f32, tag="av", bufs=2, name="ps_av")
                            for n in range(NCH):
                                nc.tensor.matmul(
                                    ps,
                                    attnT[:, n, 2 * bl : 2 * bl + 2],
                                    vts[2 * bl + n // NCHH][:, n % NCHH],
                                    start=(n == 0),
                                    stop=(n == NCH - 1),
                                )
                            sbb = work.tile([NHL, HC], f32, tag="sbb", name="sbb")
                            nc.vector.tensor_copy(sbb, ps)
                            for h in range(NHL):
                                tp = pp.tile([P, PG], f32, tag="trp", bufs=2, name="tp")
                                nc.tensor.transpose(
                                    tp[:, :NHL], sbb[:, h * P : (h + 1) * P],
                                    ident[:NHL, :NHL],
                                )
                                nc.vector.tensor_copy(
                                    aoT[:, b, h : h + 1], tp[:, h : h + 1]
                                )

                        # new-token V correction + 1/sum
                        bsl = slice(g * BG, (g + 1) * BG)
                        for h in range(NHL):
                            tmp = work.tile([P, BG], f32, tag="corr", name="tmp")
                            nc.vector.tensor_mul(tmp, vT_pairs[:, bsl, h], bcA[:, :, h])
                            nc.vector.tensor_add(
                                aoT[:, bsl, h], aoT[:, bsl, h], tmp
                            )
                            nc.vector.tensor_mul(
                                aoT[:, bsl, h], aoT[:, bsl, h], bcR[:, :, h]
                            )

                        # ---- per-group output projection + bias (each output row
                        # belongs to exactly one group, so + b_o lands once) ----
                        out_g = pers.tile([BG, H], f32, name=f"out_sb{g}")
                        for no in range(NO):
                            pso = pp.tile([BG, 512], f32, tag="av", bufs=2, name="pso")
                            for h in range(NHL):
                                nc.tensor.matmul(
                                    pso,
                                    aoT[:, bsl, h],
                                    wo_sb[:, h, no * 512 : (no + 1) * 512],
                                    start=(h == 0),
                                    stop=False,
                                )
                            nc.tensor.matmul(
                                pso, ones_r[:, :BG],
                                bo_sb[:, no * 512 : (no + 1) * 512],
                                start=False, stop=True,
                            )
                            nc.vector.tensor_copy(
                                out_g[:, no * 512 : (no + 1) * 512], pso
                            )
                        nc.sync.dma_start(out[g * BG : (g + 1) * BG, :], out_g)


        for _ in range(repeat):
            emit_body()

    nc.compile()
    return nc


def make_core_inputs(x, k_cache, v_cache, w_q, w_k, w_v, w_o, b_q, b_k, b_v, b_o,
                     current_pos, n_cores=8):
    """Host-side shard + layout prep. Returns list of per-core input dicts."""
    B, S, H = x.shape
    L = k_cache.shape[1]
    cp = int(current_pos)
    HC = H // n_cores
    NHL = HC // P
    inv = 1.0 / np.sqrt(P)

    x2 = np.ascontiguousarray(x.reshape(B, H).T, dtype=np.float32)  # [H, B]
    KCH = H // P
    NCHH = (L // P) // 2

    def wshuf(w):
        # [HC, H] weight slice -> [P, KCH, HC]: one contiguous 16 KB run per
        # partition for the projection-weight DMA
        return np.ascontiguousarray(
            np.asarray(w).T.reshape(KCH, P, HC).transpose(1, 0, 2)
        )

    def vshuf(vc):
        # [B, L, HC] -> [B, 2, P, NCHH, HC] with element (b,u,p,n,c) =
        # vc[b, (u*NCHH + n)*P + p, c]: per-partition contiguous V DMA runs
        return np.ascontiguousarray(
            np.asarray(vc).reshape(B, 2, NCHH, P, HC).transpose(0, 1, 3, 2, 4)
        )
    kT_full = np.ascontiguousarray(k_cache.transpose(0, 2, 1))      # [B, H, L]
    maskvec = np.zeros((1, L), dtype=np.float32)
    maskvec[0, cp:] = -1e30

    maps = []
    for c in range(n_cores):
        r = slice(c * HC, (c + 1) * HC)
        m = {
            "xT": x2,
            "wqT": wshuf(w_q[r, :]),
            "wkT": wshuf(w_k[r, :]),
            "wvT": wshuf(w_v[r, :]),
            "woT": np.ascontiguousarray(w_o[:, r].T),
            "bq": np.ascontiguousarray((b_q[r] * inv).reshape(NHL, P).T),
            "bk": np.ascontiguousarray(b_k[r].reshape(NHL, P).T),
            "bv": np.ascontiguousarray(b_v[r].reshape(NHL, P).T),
            "bo": (b_o if c == 0 else np.zeros_like(b_o)).reshape(1, H),
            "kT": np.ascontiguousarray(kT_full[:, r, :]),
            "v": vshuf(v_cache[:, :, r]),
            "mask": maskvec,
        }
        maps.append({k: np.asarray(a, dtype=np.float32) for k, a in m.items()})
    return maps


def kernel(x, k_cache, v_cache, w_q, w_k, w_v, w_o, b_q, b_k, b_v, b_o, current_pos):
    from concourse import bass_utils

    x, k_cache, v_cache = np.asarray(x), np.asarray(k_cache), np.asarray(v_cache)
    w_q, w_k, w_v, w_o = (np.asarray(a) for a in (w_q, w_k, w_v, w_o))
    b_q, b_k, b_v, b_o = (np.asarray(a) for a in (b_q, b_k, b_v, b_o))
    B, S, H = x.shape
    n_cores = 8
    key = (B, H, k_cache.shape[1])
    if key not in _CACHE:
        _CACHE[key] = build_nc(
            B=B, H=H, HC=H // n_cores, L=k_cache.shape[1],
            NHL=(H // n_cores) // P, n_devices=n_cores,
        )
    nc = _CACHE[key]
    in_maps = make_core_inputs(
        x, k_cache, v_cache, w_q, w_k, w_v, w_o, b_q, b_k, b_v, b_o, current_pos,
        n_cores=n_cores,
    )
    res = bass_utils.run_bass_kernel_spmd(nc, in_maps, core_ids=list(range(n_cores)))
    total = np.zeros((B, H), dtype=np.float32)
    for r in res.results:
        total += r["out"]
    return total.reshape(B, S, H).astype(np.float32)

